# revision 25
# baseline (speedup 1.0000x reference)
"""RecurrentGCN (EvolveGCN-O style) Trainium2 kernel, 8-core SPMD.

Math (reference reordered):
    W    = GRUCell(W0, W0)                     # [D,D], computed on-device
    deg  = segsum(ew+selfloops, col); dinv = deg^-1/2
    agg  = segsum(ew * (dinv*x)[row], col)     # gather + one-hot matmul scatter
    out  = dinv[:,None] * relu(agg @ W) @ W_lin.T + b_lin
           (the dinv row-scale commutes past relu/@W since dinv >= 0)

Sharding: destination nodes split into 8 contiguous ranges, one per core.
Phase A (per core, node-sharded): deg/dinv + x' = dinv*x rows (bf16).
Host between phases: concat x' table, replicate dinv (data movement only).
Phase B (per core): dma_gather x'[row] streams (sorted by dest block, split
by table half so indices fit int16), DVE builds one-hot S = (iota==dst)*ew
tiles, PE matmul-scatters agg.T += G.T @ S in PSUM per 128-dest block, then
dense head: relu(agg@W) @ W_lin.T, dinv scale, bias.
"""

import os
import sys
from contextlib import ExitStack

sys.path.insert(0, "/opt/trn_rl_repo")

import numpy as np
import ml_dtypes

import concourse.bass as bass
import concourse.mybir as mybir
from concourse.bass_utils import run_bass_kernel_spmd
from concourse.library_config import mlp
from concourse.library_overlay import lower_extended_insts

F32 = mybir.dt.float32
BF16 = mybir.dt.bfloat16
I16 = mybir.dt.int16
ALU = mybir.AluOpType
AFT = mybir.ActivationFunctionType
BF16NP = ml_dtypes.bfloat16

NCORES = 8
D = 128    # feature dim == partition width; kernel assumes D == 128
NBUF = 4   # rotating gather buffers
NS = 16    # rotating S tiles

LAST_EXEC_NS = {}  # phase -> exec_time_ns (filled when KERNEL_TRACE=1)


# ----------------------------------------------------------------------------
# planning (host, index-only)
# ----------------------------------------------------------------------------

class Plan:
    pass


def _plan(N, src, dst, blocks_per_group=7):
    """All index preprocessing. src/dst include self-loops already."""
    P = Plan()
    P.N = N
    assert N % NCORES == 0, "node count must split evenly across cores"
    P.NPC = N // NCORES
    P.NBLK = -(-P.NPC // 128)
    P.NPAD = P.NBLK * 128
    P.RT = NCORES * P.NPAD
    P.HSPLIT = P.RT // 2
    assert P.HSPLIT <= 32768 and P.RT - P.HSPLIT <= 32768, "int16 idx overflow"

    core = dst // P.NPC
    nloc = dst % P.NPC
    blk = nloc // 128
    dloc = nloc % 128
    r = (src // P.NPC) * P.NPAD + (src % P.NPC)
    par = (r >= P.HSPLIT).astype(np.int64)

    # sort edges by (core, blk, parity, r): r-sorted gathers stream HBM
    order = np.lexsort((r, par, blk, core))
    P.e_core = core[order]
    P.e_blk = blk[order]
    P.e_par = par[order]
    P.e_r = r[order]
    P.e_dloc = dloc[order]
    P.e_order = order

    key = (P.e_core * P.NBLK + P.e_blk) * 2 + P.e_par
    cnt = np.bincount(key, minlength=NCORES * P.NBLK * 2).reshape(
        NCORES, P.NBLK, 2
    )
    P.cnt = cnt
    P.bin_start = np.zeros(NCORES * P.NBLK * 2 + 1, np.int64)
    np.cumsum(cnt.reshape(-1), out=P.bin_start[1:])

    # uniform chunk counts per (blk, parity): max over cores (SPMD program)
    P.CPB = -(-cnt.max(axis=0) // 128)  # [NBLK, 2]

    GB = min(blocks_per_group, P.NBLK)
    P.groups = [
        list(range(g, min(g + GB, P.NBLK))) for g in range(0, P.NBLK, GB)
    ]
    P.segs = []    # gather calls: one per (group, parity) with edges
    P.chunks = []  # matmul schedule
    first_seen = set()
    last_chunk_of_block = {}
    off16 = 0
    for blks in P.groups:
        for par_ in (0, 1):
            nch = int(sum(P.CPB[b][par_] for b in blks))
            if nch == 0:
                continue
            k = len(P.segs)
            seg = dict(P=par_, blocks=blks, nch=nch, nk=nch * 128,
                       off16=off16, chunk0=len(P.chunks))
            P.segs.append(seg)
            jj = 0
            for b in blks:
                for _ in range(int(P.CPB[b][par_])):
                    ch = dict(seg=k, jj=jj, b=b,
                              first=(b not in first_seen),
                              last=False,
                              seg_first=(jj == 0),
                              seg_last=(jj == nch - 1))
                    first_seen.add(b)
                    last_chunk_of_block[b] = len(P.chunks)
                    P.chunks.append(ch)
                    jj += 1
            off16 += seg["nk"] // 16
    for b, j in last_chunk_of_block.items():
        P.chunks[j]["last"] = True
    P.empty_blocks = [b for b in range(P.NBLK) if b not in first_seen]
    P.NCH = len(P.chunks)
    P.NIDX16 = off16
    P.CSEG = max(s["nch"] for s in P.segs)
    P.NSEG = len(P.segs)
    done_order = [ch["b"] for ch in P.chunks if ch["last"]]
    P.done_order = done_order
    P.done_rank = {b: i for i, b in enumerate(done_order)}
    return P


def _pack_core_streams(P, c, ew_sorted):
    """Per-core gather idx / dst / ew streams in chunk layout."""
    idx = np.zeros(P.NCH * 128, np.int16)
    dstv = np.zeros((128, P.NCH), np.float32)
    eww = np.zeros((128, P.NCH), np.float32)
    chunk_off = {}
    j = 0
    for s in P.segs:
        for b in s["blocks"]:
            chunk_off[(b, s["P"])] = j
            j += int(P.CPB[b][s["P"]])
    for b in range(P.NBLK):
        for par_ in (0, 1):
            if int(P.CPB[b][par_]) == 0:
                continue
            j0 = chunk_off[(b, par_)]
            lo = P.bin_start[(c * P.NBLK + b) * 2 + par_]
            hi = P.bin_start[(c * P.NBLK + b) * 2 + par_ + 1]
            n = int(hi - lo)
            base = j0 * 128
            rr = P.e_r[lo:hi] - par_ * P.HSPLIT
            idx[base:base + n] = rr.astype(np.int16)
            pos = np.arange(n)
            dstv[pos % 128, j0 + pos // 128] = P.e_dloc[lo:hi].astype(np.float32)
            eww[pos % 128, j0 + pos // 128] = ew_sorted[lo:hi].astype(np.float32)
    # wrap idx into [128, NIDX16]: 16-lane wrap per segment, replicated x8
    idx_t = np.zeros((128, P.NIDX16), np.int16)
    for s in P.segs:
        lo = s["chunk0"] * 128
        wrap = idx[lo:lo + s["nk"]].reshape(-1, 16).T  # [16, nk/16]
        idx_t[:, s["off16"]:s["off16"] + s["nk"] // 16] = np.tile(wrap, (8, 1))
    return idx_t, dstv, eww


# ----------------------------------------------------------------------------
# phase A: deg -> dinv -> x' (node-sharded)
# ----------------------------------------------------------------------------

def _build_phase_a(P, SA):
    nc = bass.Bass(target_bir_lowering=False)
    NB, NP_ = P.NBLK, P.NPAD
    ewa = nc.dram_tensor("ewa", [128, NB * SA], F32, kind="ExternalInput")
    xa = nc.dram_tensor("xa", [128, NP_], F32, kind="ExternalInput")
    xs = nc.dram_tensor("xs", [128, NP_], BF16, kind="ExternalOutput")
    dv = nc.dram_tensor("dv", [128, NB], F32, kind="ExternalOutput")

    with ExitStack() as st:
        en = st.enter_context
        ewa_sb = en(nc.sbuf_tensor("ewa_sb", [128, NB * SA], F32))
        xa_sb = en(nc.sbuf_tensor("xa_sb", [128, NP_], F32))
        xs_sb = en(nc.sbuf_tensor("xs_sb", [128, NP_], BF16))
        deg_sb = en(nc.sbuf_tensor("deg_sb", [128, NB], F32))
        sd_sb = en(nc.sbuf_tensor("sd_sb", [128, NB], F32))
        dv_sb = en(nc.sbuf_tensor("dv_sb", [128, NB], F32))
        ld = en(nc.semaphore("a_ld"))
        s1 = en(nc.semaphore("a_s1"))
        s2 = en(nc.semaphore("a_s2"))
        s3 = en(nc.semaphore("a_s3"))
        s4 = en(nc.semaphore("a_s4"))
        so = en(nc.semaphore("a_out"))

        with nc.Block() as block:
            @block.sync
            def _(sy):
                sy.dma_start(ewa_sb[:, :], ewa[:, :]).then_inc(ld, 16)
                sy.dma_start(xa_sb[:, :], xa[:, :]).then_inc(ld, 16)
                sy.wait_ge(s3, 1)
                sy.dma_start(xs[:, :], xs_sb[:, :]).then_inc(so, 16)
                sy.dma_start(dv[:, :], dv_sb[:, :]).then_inc(so, 16)
                sy.wait_ge(so, 32)

            @block.vector
            def _(ve):
                ve.wait_ge(ld, 32)
                ve.tensor_reduce(
                    deg_sb[:, :],
                    ewa_sb[:, :].rearrange("p (b s) -> p b s", s=SA),
                    axis=mybir.AxisListType.X,
                    op=ALU.add,
                ).then_inc(s1, 1)
                ve.wait_ge(s2, 1)
                ve.reciprocal(dv_sb[:, :], sd_sb[:, :]).then_inc(s4, 1)
                ve.wait_ge(s4, 1)  # same-engine RAW barrier (dv_sb)
                for b in range(NB):
                    i = ve.tensor_scalar_mul(
                        xs_sb[:, b * 128:(b + 1) * 128],
                        xa_sb[:, b * 128:(b + 1) * 128],
                        dv_sb[:, b:b + 1],
                    )
                    if b == NB - 1:
                        i.then_inc(s3, 1)

            @block.scalar
            def _(ac):
                ac.wait_ge(s1, 1)
                ac.sqrt(sd_sb[:, :], deg_sb[:, :]).then_inc(s2, 1)
    return nc


# ----------------------------------------------------------------------------
# phase B: gather + one-hot matmul aggregation + dense head (+ GRU)
# ----------------------------------------------------------------------------

def _build_phase_b(P, has_blin):
    nc = bass.Bass(target_bir_lowering=False)
    NP_ = P.NPAD

    xprime = nc.dram_tensor("xprime", [P.RT, D], BF16, kind="ExternalInput")
    idx_t = nc.dram_tensor("idx", [128, P.NIDX16], I16, kind="ExternalInput")
    dst_t = nc.dram_tensor("dst", [128, P.NCH], F32, kind="ExternalInput")
    ew_t = nc.dram_tensor("ew", [128, P.NCH], F32, kind="ExternalInput")
    w0t_d = nc.dram_tensor("w0t", [128, 128], F32, kind="ExternalInput")
    w0_d = nc.dram_tensor("w0", [128, 128], F32, kind="ExternalInput")
    wih_d = nc.dram_tensor("wihT", [128, 384], F32, kind="ExternalInput")
    whh_d = nc.dram_tensor("whhT", [128, 384], F32, kind="ExternalInput")
    bih_d = nc.dram_tensor("bihr", [128, 384], F32, kind="ExternalInput")
    bhh_d = nc.dram_tensor("bhhr", [128, 384], F32, kind="ExternalInput")
    wlt_d = nc.dram_tensor("wlt", [128, 128], F32, kind="ExternalInput")
    blin_d = nc.dram_tensor("blin", [128, 1], F32, kind="ExternalInput")
    dvr_d = nc.dram_tensor("dinvr", [128, NP_], F32, kind="ExternalInput")
    outd = nc.dram_tensor("out", [128, NP_], F32, kind="ExternalOutput")

    # dense tiling over node columns
    dt_sizes = []
    off = 0
    while off < NP_:
        n = min(512, NP_ - off)
        dt_sizes.append((off, n))
        off += n
    NT = len(dt_sizes)

    # schedule bookkeeping for counting-sem waits
    last_chunk_of_block = {}
    last_chunk_of_seg = {}
    for j, ch in enumerate(P.chunks):
        if ch["last"]:
            last_chunk_of_block[ch["b"]] = j
        if ch["seg_last"]:
            last_chunk_of_seg[ch["seg"]] = j
    NDONE = len(P.done_order)

    with ExitStack() as st:
        en = st.enter_context
        idx_sb = en(nc.sbuf_tensor("idx_sb", [128, P.NIDX16], I16))
        dst_sb = en(nc.sbuf_tensor("dst_sb", [128, P.NCH], F32))
        ew_sb = en(nc.sbuf_tensor("ew_sb", [128, P.NCH], F32))
        iota_bf = en(nc.sbuf_tensor("iota_bf", [128, 128], F32))
        gbufs = [
            en(nc.sbuf_tensor(f"gbuf{i}", [128, P.CSEG * 128], BF16))
            for i in range(NBUF)
        ]
        s_pool = en(nc.sbuf_tensor("s_pool", [128, NS * 128], BF16))
        agg = en(nc.sbuf_tensor("agg", [128, NP_], F32))
        hrelu = en(nc.sbuf_tensor("hrelu", [128, NP_], F32))
        out_sb = en(nc.sbuf_tensor("out_sb", [128, NP_], F32))
        dvr_sb = en(nc.sbuf_tensor("dvr_sb", [128, NP_], F32))
        w0t_sb = en(nc.sbuf_tensor("w0t_sb", [128, 128], F32))
        w0_sb = en(nc.sbuf_tensor("w0_sb", [128, 128], F32))
        wih_sb = en(nc.sbuf_tensor("wih_sb", [128, 384], F32))
        whh_sb = en(nc.sbuf_tensor("whh_sb", [128, 384], F32))
        bih_sb = en(nc.sbuf_tensor("bih_sb", [128, 384], F32))
        bhh_sb = en(nc.sbuf_tensor("bhh_sb", [128, 384], F32))
        wlt_sb = en(nc.sbuf_tensor("wlt_sb", [128, 128], F32))
        blin_sb = en(nc.sbuf_tensor("blin_sb", [128, 1], F32))
        wn_sb = en(nc.sbuf_tensor("wn_sb", [128, 128], F32))
        g_r = en(nc.sbuf_tensor("g_r", [128, 128], F32))
        g_z = en(nc.sbuf_tensor("g_z", [128, 128], F32))
        g_t1 = en(nc.sbuf_tensor("g_t1", [128, 128], F32))
        g_t2 = en(nc.sbuf_tensor("g_t2", [128, 128], F32))
        g_n = en(nc.sbuf_tensor("g_n", [128, 128], F32))

        ps = en(nc.psum_tensor("ps", [128, 8 * 512], F32))
        ps_r = ps[:, 4 * 512:4 * 512 + 128]
        ps_z = ps[:, 4 * 512 + 128:4 * 512 + 256]
        ps_in = ps[:, 5 * 512:5 * 512 + 128]
        ps_hn = ps[:, 5 * 512 + 128:5 * 512 + 256]

        lds = [en(nc.semaphore(f"ld{i}")) for i in range(11)]
        io_s = en(nc.semaphore("io_s"))
        g_done = [en(nc.semaphore(f"g_done{i}")) for i in range(NBUF)]
        s_ready = en(nc.semaphore("s_ready"))   # DVE S tiles, +1 per chunk
        mm_done = en(nc.semaphore("mm_done"))   # PE agg matmuls, +1 per chunk
        ev_done = en(nc.semaphore("ev_done"))   # ACT evictions, +1 per block
        vsync = en(nc.semaphore("vsync"))       # DVE GRU/setup op counter
        gru_r = en(nc.semaphore("gru_r"))
        gru_z = en(nc.semaphore("gru_z"))
        gru_n = en(nc.semaphore("gru_n"))
        gru_mm = en(nc.semaphore("gru_mm"))
        d1_ready = en(nc.semaphore("d1_ready"))
        d2_ready = en(nc.semaphore("d2_ready"))
        r_done = en(nc.semaphore("r_done"))     # DVE relu, +1 per tile
        f_done = en(nc.semaphore("f_done"))     # DVE final mult, +1 per tile
        fin = en(nc.semaphore("fin"))           # bias add (has_blin only)
        osem = en(nc.semaphore("osem"))
        dvr_ld = en(nc.semaphore("dvr_ld"))
        mz = en(nc.semaphore("mz"))

        # DVE GRU op count thresholds (vsync values)
        V_R = 2    # g_t1 pre-act ready for ACT sigmoid
        V_Z = 4    # g_t2 pre-act ready
        V_PS = 7   # all GRU psum reads done (banks 4,5 reusable)
        V_N = 8    # g_n pre-act ready for ACT tanh
        V_WN = 12  # wn_sb ready

        loads = [
            (idx_sb, idx_t), (dst_sb, dst_t), (ew_sb, ew_t),
            (w0t_sb, w0t_d), (wih_sb, wih_d), (whh_sb, whh_d),
            (w0_sb, w0_d), (bih_sb, bih_d), (bhh_sb, bhh_d),
            (wlt_sb, wlt_d), (blin_sb, blin_d),
        ]

        with nc.Block() as block:
            @block.sync
            def _(sy):
                for li, (sbt, drt) in enumerate(loads):
                    sy.dma_start(sbt[:, :], drt[:, :]).then_inc(lds[li], 16)
                # defer the big dinv_rep tile until the gather storm wanes
                sy.wait_ge(ev_done, max(1, NDONE - 8))
                sy.dma_start(dvr_sb[:, :], dvr_d[:, :]).then_inc(dvr_ld, 16)
                out_gate = fin if has_blin else f_done
                for t, (o, n) in enumerate(dt_sizes):
                    sy.wait_ge(out_gate, t + 1)
                    sy.dma_start(
                        outd[:, o:o + n], out_sb[:, o:o + n]
                    ).then_inc(osem, 16)
                sy.wait_ge(osem, 16 * NT)

            @block.gpsimd
            def _(gp):
                gp.iota(
                    iota_bf[:, :], [[1, 128]], channel_multiplier=0,
                    allow_small_or_imprecise_dtypes=True,
                ).then_inc(io_s, 1)
                gp.load_library(mlp)
                gp.wait_ge(lds[0], 16)
                for k, seg in enumerate(P.segs):
                    if k >= NBUF:
                        gp.wait_ge(mm_done,
                                   last_chunk_of_seg[k - NBUF] + 1)
                    if seg["P"] == 0:
                        src_ap = xprime[0:P.HSPLIT, :]
                    else:
                        src_ap = xprime[P.HSPLIT:P.RT, :]
                    out_ap = gbufs[k % NBUF][:, 0:seg["nch"] * 128].rearrange(
                        "p (c f) -> p c f", f=128
                    )
                    gp.dma_gather(
                        out_ap,
                        src_ap,
                        idx_sb[:, seg["off16"]:seg["off16"] + seg["nk"] // 16],
                        seg["nk"],
                        seg["nk"],
                        128,
                        single_packet=False,
                    ).then_inc(g_done[k % NBUF], 16)

            @block.tensor
            def _(pe):
                # --- GRU matmuls: all six share stationary lhsT = W0.T
                for li in (3, 4, 5):
                    pe.wait_ge(lds[li], 16)
                pe.matmul(ps_r, w0t_sb[:, :], wih_sb[:, 0:128],
                          start=True, stop=False, skip_group_check=True)
                pe.matmul(ps_r, w0t_sb[:, :], whh_sb[:, 0:128],
                          start=False, stop=True, skip_group_check=True)
                pe.matmul(ps_z, w0t_sb[:, :], wih_sb[:, 128:256],
                          start=True, stop=False, skip_group_check=True)
                pe.matmul(ps_z, w0t_sb[:, :], whh_sb[:, 128:256],
                          start=False, stop=True, skip_group_check=True)
                pe.matmul(ps_in, w0t_sb[:, :], wih_sb[:, 256:384],
                          start=True, stop=True, skip_group_check=True)
                pe.matmul(ps_hn, w0t_sb[:, :], whh_sb[:, 256:384],
                          start=True, stop=True,
                          skip_group_check=True).then_inc(gru_mm, 1)

                # --- aggregation
                gru_guard_done = False
                for j, ch in enumerate(P.chunks):
                    k = ch["seg"]
                    if ch["seg_first"]:
                        pe.wait_ge(g_done[k % NBUF], 16 * (k // NBUF + 1))
                    if ch["first"]:
                        rank = P.done_rank[ch["b"]]
                        slot = ch["b"] % 8
                        prior = [bb for bb in P.done_order[:rank]
                                 if bb % 8 == slot]
                        if prior:
                            pe.wait_ge(ev_done, P.done_rank[prior[-1]] + 1)
                        if ch["b"] >= 4 and not gru_guard_done:
                            pe.wait_ge(vsync, V_PS)
                            gru_guard_done = True
                    pe.wait_ge(s_ready, j + 1)
                    pe.matmul(
                        ps[:, (ch["b"] % 8) * 512:(ch["b"] % 8) * 512 + 128],
                        gbufs[k % NBUF][:, ch["jj"] * 128:(ch["jj"] + 1) * 128],
                        s_pool[:, (j % NS) * 128:(j % NS) * 128 + 128],
                        start=ch["first"], stop=ch["last"],
                        skip_group_check=True,
                    ).then_inc(mm_done, 1)

                # --- dense: h.T = W.T @ agg ; out.T = W_lin @ relu(h.T)
                # all PSUM banks were aggregation slots: wait for the
                # final eviction before dense matmuls reuse them
                pe.wait_ge(vsync, V_WN)
                pe.wait_ge(ev_done, NDONE)
                if P.empty_blocks:
                    pe.wait_ge(mz, 1)
                for t, (o, n) in enumerate(dt_sizes):
                    if t >= 4:
                        pe.wait_ge(r_done, t - 3)
                    pe.matmul(
                        ps[:, (t % 4) * 512:(t % 4) * 512 + n],
                        wn_sb[:, :], agg[:, o:o + n],
                        start=True, stop=True, skip_group_check=True,
                    ).then_inc(d1_ready, 1)
                pe.wait_ge(lds[9], 16)
                for t, (o, n) in enumerate(dt_sizes):
                    pe.wait_ge(r_done, t + 1)
                    if t >= 4:
                        pe.wait_ge(f_done, t - 3)
                    pe.matmul(
                        ps[:, (4 + t % 4) * 512:(4 + t % 4) * 512 + n],
                        wlt_sb[:, :], hrelu[:, o:o + n],
                        start=True, stop=True, skip_group_check=True,
                    ).then_inc(d2_ready, 1)

            @block.vector
            def _(ve):
                vb = [0]

                def vstep(inst):
                    # every tracked DVE op bumps vsync; cross/self waits use
                    # the python-side counter values
                    vb[0] += 1
                    inst.then_inc(vsync, 1)

                def vbar():
                    ve.wait_ge(vsync, vb[0])

                # zero agg columns of blocks that receive no edges anywhere
                if P.empty_blocks:
                    for i, b in enumerate(P.empty_blocks):
                        m = ve.memset(agg[:, b * 128:(b + 1) * 128], 0)
                        if i == len(P.empty_blocks) - 1:
                            m.then_inc(mz, 1)

                # --- GRU elementwise (12 vsync-counted ops)
                ve.wait_ge(gru_mm, 1)
                for li in (6, 7, 8, 10):
                    ve.wait_ge(lds[li], 16)
                vstep(ve.tensor_tensor(g_t1[:, :], ps_r, bih_sb[:, 0:128],
                                       op=ALU.add))                      # 1
                vbar()
                vstep(ve.tensor_tensor(g_t1[:, :], g_t1[:, :],
                                       bhh_sb[:, 0:128], op=ALU.add))    # 2
                vstep(ve.tensor_tensor(g_t2[:, :], ps_z, bih_sb[:, 128:256],
                                       op=ALU.add))                      # 3
                vbar()
                vstep(ve.tensor_tensor(g_t2[:, :], g_t2[:, :],
                                       bhh_sb[:, 128:256], op=ALU.add))  # 4
                vstep(ve.tensor_tensor(g_n[:, :], ps_hn, bhh_sb[:, 256:384],
                                       op=ALU.add))                      # 5
                vbar()
                ve.wait_ge(gru_r, 1)  # g_r written by ACT sigmoid
                vstep(ve.tensor_tensor(g_n[:, :], g_n[:, :], g_r[:, :],
                                       op=ALU.mult))                     # 6
                vbar()
                vstep(ve.tensor_tensor(g_n[:, :], g_n[:, :], ps_in,
                                       op=ALU.add))                      # 7
                vbar()
                vstep(ve.tensor_tensor(g_n[:, :], g_n[:, :],
                                       bih_sb[:, 256:384], op=ALU.add))  # 8
                # W_new = n - z*n + z*W0   (g_n holds tanh(n) after ACT)
                ve.wait_ge(gru_n, 1)
                ve.wait_ge(gru_z, 1)
                vstep(ve.tensor_tensor(g_t1[:, :], g_z[:, :], w0_sb[:, :],
                                       op=ALU.mult))                     # 9
                vstep(ve.tensor_tensor(g_t2[:, :], g_z[:, :], g_n[:, :],
                                       op=ALU.mult))                     # 10
                vbar()
                vstep(ve.tensor_tensor(wn_sb[:, :], g_n[:, :], g_t2[:, :],
                                       op=ALU.subtract))                 # 11
                vbar()
                vstep(ve.tensor_tensor(wn_sb[:, :], wn_sb[:, :],
                                       g_t1[:, :], op=ALU.add))          # 12
                assert vb[0] == V_WN

                # --- S tiles
                ve.wait_ge(io_s, 1)
                ve.wait_ge(lds[1], 16)
                ve.wait_ge(lds[2], 16)
                for j in range(P.NCH):
                    if j >= NS:
                        ve.wait_ge(mm_done, j - NS + 1)
                    ve.tensor_scalar(
                        s_pool[:, (j % NS) * 128:(j % NS) * 128 + 128],
                        iota_bf[:, :],
                        dst_sb[:, j:j + 1],
                        ew_sb[:, j:j + 1],
                        op0=ALU.is_equal,
                        op1=ALU.mult,
                    ).then_inc(s_ready, 1)

                # --- dense epilogue
                ve.wait_ge(dvr_ld, 16)
                for t, (o, n) in enumerate(dt_sizes):
                    ve.wait_ge(d1_ready, t + 1)
                    ve.tensor_scalar_max(
                        hrelu[:, o:o + n],
                        ps[:, (t % 4) * 512:(t % 4) * 512 + n],
                        0.0,
                    ).then_inc(r_done, 1)
                for t, (o, n) in enumerate(dt_sizes):
                    ve.wait_ge(d2_ready, t + 1)
                    ve.tensor_tensor(
                        out_sb[:, o:o + n],
                        ps[:, (4 + t % 4) * 512:(4 + t % 4) * 512 + n],
                        dvr_sb[:, o:o + n],
                        op=ALU.mult,
                    ).then_inc(f_done, 1)
                    if has_blin:
                        ve.wait_ge(f_done, t + 1)
                        ve.tensor_scalar_add(
                            out_sb[:, o:o + n], out_sb[:, o:o + n],
                            blin_sb[:, 0:1],
                        ).then_inc(fin, 1)

            @block.scalar
            def _(ac):
                ac.wait_ge(vsync, V_R)
                ac.activation(g_r[:, :], g_t1[:, :],
                              AFT.Sigmoid).then_inc(gru_r, 1)
                ac.wait_ge(vsync, V_Z)
                ac.activation(g_z[:, :], g_t2[:, :],
                              AFT.Sigmoid).then_inc(gru_z, 1)
                ac.wait_ge(vsync, V_N)
                ac.activation(g_n[:, :], g_n[:, :],
                              AFT.Tanh).then_inc(gru_n, 1)
                # --- PSUM -> agg evictions (ACT is otherwise idle here)
                for rank, b in enumerate(P.done_order):
                    ac.wait_ge(mm_done, last_chunk_of_block[b] + 1)
                    ac.activation(
                        agg[:, b * 128:(b + 1) * 128],
                        ps[:, (b % 8) * 512:(b % 8) * 512 + 128],
                        AFT.Copy,
                    ).then_inc(ev_done, 1)

    lower_extended_insts(nc)
    return nc


# ----------------------------------------------------------------------------
# top level
# ----------------------------------------------------------------------------

def _run(nc, in_maps, phase):
    if os.environ.get("KERNEL_BACKEND") == "sim":
        from concourse.bass_interp import CoreSim
        results = []
        for m in in_maps:
            sim = CoreSim(nc)
            for k, v in m.items():
                sim.tensor(k)[:] = np.asarray(v).reshape(sim.tensor(k).shape)
            sim.simulate()
            outs = {}
            for alloc in nc.m.functions[0].allocations:
                if getattr(alloc, "kind", None) == "ExternalOutput":
                    name = alloc.memorylocations[0].name
                    outs[name] = np.array(sim.tensor(name))
            results.append(outs)
        return results
    trace = bool(int(os.environ.get("KERNEL_TRACE", "0")))
    if trace:
        try:
            _install_ntff_hook()
            res = run_bass_kernel_spmd(
                nc, in_maps, core_ids=list(range(NCORES)), trace=True,
                trace_cores=[0],
            )
            LAST_EXEC_NS[phase] = res.exec_time_ns
            LAST_TRACE[phase] = res.instructions_and_trace
            return res.results
        except Exception as e:  # degrade to an untraced run
            print(f"trace run failed ({type(e).__name__}: {e}); retrying",
                  file=sys.stderr)
    res = run_bass_kernel_spmd(
        nc, in_maps, core_ids=list(range(NCORES)), trace=False
    )
    return res.results


LAST_TRACE = {}


def _install_ntff_hook():
    """The agent image lacks antenv.axon_hooks; synthesize it and install
    the ctypes NTFF hook the axon boot would have registered."""
    import types
    import antenv
    if "antenv.axon_hooks" not in sys.modules:
        mod = types.ModuleType("antenv.axon_hooks")
        mod._hook = None

        def set_axon_ntff_profile_hook(h):
            mod._hook = h

        def get_axon_ntff_profile_hook():
            return mod._hook

        mod.set_axon_ntff_profile_hook = set_axon_ntff_profile_hook
        mod.get_axon_ntff_profile_hook = get_axon_ntff_profile_hook
        sys.modules["antenv.axon_hooks"] = mod
        antenv.axon_hooks = mod
    mod = sys.modules["antenv.axon_hooks"]
    if mod._hook is None:
        sys.path.insert(0, "/root/.axon_site")
        from trn_agent_boot.trn_boot import _ntff_profile_via_ctypes
        mod._hook = _ntff_profile_via_ctypes("/opt/axon/libaxon_pjrt.so")


def kernel(x, edge_index, edge_weight, W0, w_ih, w_hh, b_ih, b_hh,
           W_lin, b_lin):
    x = np.asarray(x, np.float32)
    edge_index = np.asarray(edge_index).astype(np.int64)
    ew = np.asarray(edge_weight, np.float32)
    W0 = np.ascontiguousarray(np.asarray(W0, np.float32))
    w_ih = np.asarray(w_ih, np.float32)
    w_hh = np.asarray(w_hh, np.float32)
    b_ih = np.asarray(b_ih, np.float32)
    b_hh = np.asarray(b_hh, np.float32)
    W_lin = np.asarray(W_lin, np.float32)
    b_lin = np.asarray(b_lin, np.float32)

    N = x.shape[0]
    # append self loops (weight 1.0)
    src = np.concatenate([edge_index[0], np.arange(N, dtype=np.int64)])
    dst = np.concatenate([edge_index[1], np.arange(N, dtype=np.int64)])
    wall = np.concatenate([ew, np.ones(N, np.float32)])

    P = _plan(N, src, dst)
    ew_sorted = wall[P.e_order]

    # ---- phase A inputs: per-dest slot layout for ew
    t_order = np.argsort(dst, kind="stable")
    t_sorted = dst[t_order]
    w_sorted = wall[t_order]
    starts = np.searchsorted(t_sorted, np.arange(N))
    slot = np.arange(len(t_sorted)) - starts[t_sorted]
    SA = int(slot.max()) + 1
    ewa = np.zeros((NCORES, 128, P.NBLK, SA), np.float32)
    cc = t_sorted // P.NPC
    nn = t_sorted % P.NPC
    ewa[cc, nn % 128, nn // 128, slot] = w_sorted
    for n in range(P.NPC, P.NPAD):   # dummy rows: deg 1
        ewa[:, n % 128, n // 128, 0] = 1.0

    xp = np.zeros((NCORES, P.NPAD, D), np.float32)
    for c in range(NCORES):
        xp[c, :P.NPC] = x[c * P.NPC:(c + 1) * P.NPC]
    xa = np.ascontiguousarray(
        xp.reshape(NCORES, P.NBLK, 128, D).transpose(0, 2, 1, 3)
    ).reshape(NCORES, 128, P.NBLK * D)

    ncA = _build_phase_a(P, SA)
    in_maps_a = [
        {"ewa": np.ascontiguousarray(ewa[c].reshape(128, P.NBLK * SA)),
         "xa": xa[c]}
        for c in range(NCORES)
    ]
    resA = _run(ncA, in_maps_a, "A")

    # ---- host shuffle between phases (pure data movement)
    xprime = np.empty((P.RT, D), BF16NP)
    dinv_rep = []
    for c in range(NCORES):
        xs = np.asarray(resA[c]["xs"]).astype(BF16NP).reshape(128, P.NBLK, D)
        xprime[c * P.NPAD:(c + 1) * P.NPAD] = (
            xs.transpose(1, 0, 2).reshape(P.NPAD, D)
        )
        dvc = np.asarray(resA[c]["dv"], np.float32).reshape(128, P.NBLK)
        dinv_vec = dvc.T.reshape(P.NPAD)
        dinv_rep.append(np.ascontiguousarray(
            np.broadcast_to(dinv_vec[None, :], (128, P.NPAD))))

    # ---- phase B
    ncB = _build_phase_b(P, bool(np.any(b_lin)))
    w0t = np.ascontiguousarray(W0.T)
    wihT = np.ascontiguousarray(w_ih.T)
    whhT = np.ascontiguousarray(w_hh.T)
    bihr = np.ascontiguousarray(np.broadcast_to(b_ih[None, :], (128, 3 * D)))
    bhhr = np.ascontiguousarray(np.broadcast_to(b_hh[None, :], (128, 3 * D)))
    wlt = np.ascontiguousarray(W_lin.T)
    blin = np.ascontiguousarray(b_lin.reshape(128, 1))

    in_maps_b = []
    for c in range(NCORES):
        idxc, dstv, eww = _pack_core_streams(P, c, ew_sorted)
        in_maps_b.append({
            "xprime": xprime, "idx": idxc, "dst": dstv, "ew": eww,
            "w0t": w0t, "w0": W0, "wihT": wihT, "whhT": whhT,
            "bihr": bihr, "bhhr": bhhr, "wlt": wlt, "blin": blin,
            "dinvr": dinv_rep[c],
        })
    resB = _run(ncB, in_maps_b, "B")

    out = np.empty((N, D), np.float32)
    for c in range(NCORES):
        oc = np.asarray(resB[c]["out"], np.float32).reshape(128, P.NPAD)
        out[c * P.NPC:(c + 1) * P.NPC] = oc.T[:P.NPC]
    return out
